# revision 13
# baseline (speedup 1.0000x reference)
"""CaptionNet Trainium2 kernel (8-core SPMD, data-parallel over batch).

Per core (batch shard Bc=32): attention-LSTM recurrence fully on-chip in a
feature-on-partition / batch-on-free layout, bf16 matmul operands with fp32
PSUM accumulation.

v2 design (vs the first working version):
- The vocab projection is interleaved into the recurrence as PE filler work
  (5 (m-chunk, v-chunk) slots per step from t=4 on), which removes the
  serial vocab tail and keeps the tensor engine continuously busy so it
  ramps to the full clock. Half of vocab_W.T stays resident in SBUF; the
  other half streams from HBM once per m-chunk.
- exp(z) is computed as sigmoid(z)/sigmoid(-z) (reciprocal on DVE), so every
  scalar-engine activation (sigmoid/tanh/relu/copy) lives in one activation
  table and the per-step 1.28us ACT_TABLE_LOAD swaps disappear.
- The softmax normalizer 1/Z is broadcast across partitions by the PE
  (ones-column x recip-row) and applied after the ctx transpose with one DVE
  multiply, keeping it off the softmax->einsum critical path.
- gamma is computed in C-major layout (16 LDWEIGHTS-light matmuls with N=32
  instead of 5 N=512 streams) and applied post-transpose.
- The attention einsum runs 4 concurrent PE column strips (8 rounds of 4
  samples); round extraction copies rotate over scalar/vector/gpsimd.
- Logits are written bf16 and upcast on the host.
"""

import numpy as np
import ml_dtypes

import concourse.bass as bass
import concourse.tile as tile
import concourse.mybir as mybir

BF16 = mybir.dt.bfloat16
F32 = mybir.dt.float32
AF = mybir.ActivationFunctionType
OP = mybir.AluOpType

# Problem constants (full size)
B_FULL, T_FULL, H, WV, F, C, V_FULL = 256, 20, 512, 301, 196, 512, 9871
N_CORES = 8
F_HI = 128
F_LO = F - F_HI  # 68

VCHUNK = 512
N_VRES = 10  # resident v-chunks (first N_VRES*512 vocab columns stay in SBUF)


def _tiles(total, step=128):
    return [(i, min(step, total - i)) for i in range(0, total, step)]


def build_program(Bc=32, T=20, V=V_FULL, stage=99):
    TB = Bc * T
    NG = Bc // 8  # sample groups of 8 (= einsum strips)
    NR = Bc // 4  # einsum rounds per step (4 strips)
    nc = bass.Bass()

    # ---------------- DRAM I/O (per-core) ----------------
    encT_d = nc.dram_tensor("encT", [Bc, F, C], BF16, kind="ExternalInput")
    xT_d = nc.dram_tensor("xT", [WV, TB], BF16, kind="ExternalInput")
    AxT_d = nc.dram_tensor("AxT", [WV, F], BF16, kind="ExternalInput")
    AhT_d = nc.dram_tensor("AhT", [H, F], BF16, kind="ExternalInput")
    WxT_d = nc.dram_tensor("WxT", [WV, WV], BF16, kind="ExternalInput")
    WcT_d = nc.dram_tensor("WcT", [C, WV], BF16, kind="ExternalInput")
    gateTa_d = nc.dram_tensor("gateTa", [H + 1, C], BF16, kind="ExternalInput")
    WihT_d = nc.dram_tensor("WihT", [WV, 4 * H], BF16, kind="ExternalInput")
    WhhT_d = nc.dram_tensor("WhhT", [H, 4 * H], BF16, kind="ExternalInput")
    vWT_d = nc.dram_tensor("vWT", [H, V], BF16, kind="ExternalInput")
    attnb_d = nc.dram_tensor("attn_br", [1, F], BF16, kind="ExternalInput")
    combb_d = nc.dram_tensor("comb_br", [1, WV], BF16, kind="ExternalInput")
    lstmb_d = nc.dram_tensor("lstm_bc", [128, 16, Bc], F32, kind="ExternalInput")
    eye_d = nc.dram_tensor("eye", [Bc, Bc], BF16, kind="ExternalInput")
    onesc_d = nc.dram_tensor("ones_col", [F, 1], BF16, kind="ExternalInput")
    onesr_d = nc.dram_tensor("ones_row", [1, TB], BF16, kind="ExternalInput")
    onesf_d = nc.dram_tensor("ones_f32", [1, 128], F32, kind="ExternalInput")
    out_d = nc.dram_tensor("out", [TB, V], BF16, kind="ExternalOutput")

    wv_t = _tiles(WV)   # [(0,128),(128,128),(256,45)]
    h_t = _tiles(H)     # 4 x 128
    f_t = [(0, F_HI), (F_HI, F_LO)]
    NWV, NH, NF = len(wv_t), len(h_t), len(f_t)
    n_mv = _tiles(TB)   # vocab m-chunks along T*Bc (5 x 128)
    v_ch = _tiles(V, VCHUNK)  # vocab n-chunks (20)

    with tile.TileContext(nc) as tc:
        with (
            tc.tile_pool(name="w", bufs=1) as wp,
            tc.tile_pool(name="act", bufs=2) as ap,
            tc.tile_pool(name="big", bufs=1) as bp,
            tc.tile_pool(name="st", bufs=2) as st,
            tc.tile_pool(name="vs", bufs=2) as vsp,
            tc.tile_pool(name="vo", bufs=2) as vp,
            tc.tile_pool(name="psE", bufs=3, space="PSUM") as psE,
            tc.tile_pool(name="psV", bufs=2, space="PSUM") as psV,
            tc.tile_pool(name="psS", bufs=1, space="PSUM") as psS,
        ):

            # ---------------- resident loads ----------------
            def load_ktiles(dram, ktiles, ncols, dt, name):
                out = []
                for ki, (k0, ks) in enumerate(ktiles):
                    tl = wp.tile([ks, ncols], dt, tag=f"{name}{ki}", name=f"{name}{ki}")
                    nc.sync.dma_start(tl[:], dram[k0 : k0 + ks, :])
                    out.append(tl)
                return out

            ones_f = load_ktiles(onesc_d, f_t, 1, BF16, "ones")
            ones1 = wp.tile([1, TB], BF16, tag="onesr", name="onesr")
            nc.sync.dma_start(ones1[:], onesr_d[:])
            onesf32 = wp.tile([1, 128], F32, tag="onesf32", name="onesf32")
            nc.sync.dma_start(onesf32[:], onesf_d[:])
            eye_sb = wp.tile([Bc, Bc], BF16, tag="eye", name="eye")
            nc.sync.dma_start(eye_sb[:], eye_d[:])

            xT_sb = load_ktiles(xT_d, wv_t, TB, BF16, "xT")
            AxT_sb = load_ktiles(AxT_d, wv_t, F, BF16, "AxT")
            AhT_sb = load_ktiles(AhT_d, h_t, F, BF16, "AhT")
            WxT_sb = load_ktiles(WxT_d, wv_t, WV, BF16, "WxT")
            WcT_sb = load_ktiles(WcT_d, h_t, WV, BF16, "WcT")
            gateT_sb = load_ktiles(gateTa_d, h_t, C, BF16, "gateT")
            gateB_sb = wp.tile([1, C], BF16, tag="gateB", name="gateB")
            nc.sync.dma_start(gateB_sb[:], gateTa_d[H : H + 1, :])
            WihT_sb = load_ktiles(WihT_d, wv_t, 4 * H, BF16, "WihT")
            WhhT_sb = load_ktiles(WhhT_d, h_t, 4 * H, BF16, "WhhT")
            attnb_sb = wp.tile([1, F], BF16, tag="attnbr", name="attnbr")
            nc.sync.dma_start(attnb_sb[:], attnb_d[:])
            combb_sb = wp.tile([1, WV], BF16, tag="combbr", name="combbr")
            nc.sync.dma_start(combb_sb[:], combb_d[:])
            lstmb_sb = wp.tile([128, 16, Bc], F32, tag="lstmb", name="lstmb")
            nc.sync.dma_start(lstmb_sb[:], lstmb_d[:])

            # encoding, split per f-tile x sample-group for fine-grained deps
            encT_r = encT_d.rearrange("b f c -> f b c")
            enc_sb = [[None] * NG for _ in range(NF)]
            for fi, (f0, fs) in enumerate(f_t):
                for g in range(NG):
                    e = wp.tile([fs, 8, C], BF16, tag=f"enc{fi}_{g}", name=f"enc{fi}_{g}")
                    nc.sync.dma_start(e[:], encT_r[f0 : f0 + fs, 8 * g : 8 * g + 8, :])
                    enc_sb[fi][g] = e

            # resident half of the vocab weights [128, NH, N_VRES*VCHUNK]
            VRES = N_VRES * VCHUNK
            vres = wp.tile([128, NH, VRES], BF16, tag="vres", name="vres")
            for ki in range(NH):
                nc.sync.dma_start(
                    vres[:, ki, :], vWT_d[128 * ki : 128 * ki + 128, 0:VRES]
                )

            h_all = bp.tile([128, NH, TB], BF16, tag="h_all", name="h_all")

            if stage < 1:
                return nc

            # ---------------- x-precomputes: zx = x@Ax.T+b ; cx = x@Wx.T+b ----
            def precompute(weights, mtiles, dst_tiles, bias_row):
                for mi, (m0, ms) in enumerate(mtiles):
                    for n0 in range(0, TB, 512):
                        nn = min(512, TB - n0)
                        pps = psE.tile([128, 512], F32, tag="ein", name="pre")
                        for ki in range(len(weights)):
                            nc.tensor.matmul(
                                pps[0:ms, 0:nn],
                                weights[ki][:, m0 : m0 + ms],
                                xT_sb[ki][:, n0 : n0 + nn],
                                start=(ki == 0),
                                stop=False,
                            )
                        nc.tensor.matmul(
                            pps[0:ms, 0:nn],
                            bias_row[:, m0 : m0 + ms],
                            ones1[:, n0 : n0 + nn],
                            start=False,
                            stop=True,
                        )
                        nc.vector.tensor_copy(
                            dst_tiles[mi][:, n0 : n0 + nn], pps[0:ms, 0:nn]
                        )

            zx_sb = [
                bp.tile([fs, TB], BF16, tag=f"zx{fi}", name=f"zx{fi}")
                for fi, (f0, fs) in enumerate(f_t)
            ]
            precompute(AxT_sb, f_t, zx_sb, attnb_sb)
            cx_sb = [
                bp.tile([ms, TB], BF16, tag=f"cx{mi}", name=f"cx{mi}")
                for mi, (m0, ms) in enumerate(wv_t)
            ]
            precompute(WxT_sb, wv_t, cx_sb, combb_sb)

            if stage < 2:
                return nc

            # ---------------- vocab projection slots ----------------
            # Work item (j, v): out[m-chunk j, v-chunk v]. m-chunk j (tokens
            # 128j..128j+128 = steps 4j..4j+4) is ready after step 4j+3, so
            # emitting items 5(t-4)..5(t-4)+5 at step t keeps every item legal.
            vcopy_rr = [0]

            def vocab_slot(j, v):
                m0, ms = n_mv[j]
                n0, nn = v_ch[v]
                if v < N_VRES:
                    vw_ap = vres[:, :, n0 : n0 + nn]
                else:
                    vst = vsp.tile([128, NH, VCHUNK], BF16, tag="vs", name="vs")
                    for ki in range(NH):
                        nc.sync.dma_start(
                            vst[:, ki, 0:nn],
                            vWT_d[128 * ki : 128 * ki + 128, n0 : n0 + nn],
                        )
                    vw_ap = vst[:, :, :]
                vps = psV.tile([128, 512], F32, tag="voc", name="voc")
                for ki in range(NH):
                    nc.tensor.matmul(
                        vps[0:ms, 0:nn],
                        h_all[:, ki, m0 : m0 + ms],
                        vw_ap[:, ki, 0:nn],
                        start=(ki == 0),
                        stop=(ki == NH - 1),
                    )
                vo = vp.tile([128, 512], BF16, tag="vout", name="vout")
                r = vcopy_rr[0] % 2
                vcopy_rr[0] += 1
                if r == 0:
                    nc.scalar.activation(vo[0:ms, 0:nn], vps[0:ms, 0:nn], AF.Copy)
                else:
                    nc.vector.tensor_copy(vo[0:ms, 0:nn], vps[0:ms, 0:nn])
                nc.sync.dma_start(out_d[m0 : m0 + ms, n0 : n0 + nn], vo[0:ms, 0:nn])

            vitems = [(j, v) for j in range(len(n_mv)) for v in range(len(v_ch))]
            vpos = [0]  # next unemitted item

            def emit_vocab(k):
                for _ in range(k):
                    if vpos[0] < len(vitems):
                        j, v = vitems[vpos[0]]
                        vocab_slot(j, v)
                        vpos[0] += 1

            # ---------------- recurrence ----------------
            c_prev = None
            h_prev = None
            for t in range(T):
                tc0, tc1 = t * Bc, (t + 1) * Bc

                # vocab filler A: runs on PE while the previous step's LSTM
                # elementwise chain produces h_{t-1} (emitted first so the PE
                # is not blocked behind zh's wait).
                if t >= 4:
                    emit_vocab(3)

                # one packed PSUM bank for the step's small tensors:
                # [0:2]=zh, [2:6]=gamT, [6:10]=se4 (row 0) then rbc (all rows,
                # written only after se4 is consumed)
                sm = psS.tile([128, 10, Bc], F32, tag="small", name="sm")
                zh_ps = sm[:, 0:2, :]
                gam_ps = sm[:, 2:6, :]

                # attention z, h-part
                if t > 0:
                    for mi, (m0, ms) in enumerate(f_t):
                        for ki in range(NH):
                            nc.tensor.matmul(
                                zh_ps[0:ms, mi, :],
                                AhT_sb[ki][:, m0 : m0 + ms],
                                h_prev[ki],
                                start=(ki == 0),
                                stop=(ki == NH - 1),
                            )

                # gamma pre-activation, C-major: gamT[c, b]
                for m in range(NH):
                    if t > 0:
                        for ki in range(NH):
                            nc.tensor.matmul(
                                gam_ps[:, m, :],
                                gateT_sb[ki][:, 128 * m : 128 * m + 128],
                                h_prev[ki],
                                start=(ki == 0),
                                stop=False,
                            )
                    nc.tensor.matmul(
                        gam_ps[:, m, :],
                        gateB_sb[:, 128 * m : 128 * m + 128],
                        ones1[:, 0:Bc],
                        start=(t == 0),
                        stop=True,
                    )
                del m

                # expz = exp(zx + zh) via sigmoid(z)/sigmoid(-z); bf16 [fs,1,Bc]
                expz = []
                for fi, (f0, fs) in enumerate(f_t):
                    if t > 0:
                        zs = ap.tile([fs, Bc], F32, tag=f"zs{fi}", name=f"zs{fi}")
                        nc.vector.tensor_tensor(
                            zs[:], zh_ps[0:fs, fi, :],
                            zx_sb[fi][:, tc0:tc1], op=OP.add,
                        )
                        src = zs[:, :]
                    else:
                        src = zx_sb[fi][:, tc0:tc1]
                    sp = ap.tile([fs, Bc], F32, tag=f"sp{fi}", name=f"sp{fi}")
                    sn = ap.tile([fs, Bc], F32, tag=f"sn{fi}", name=f"sn{fi}")
                    nc.scalar.activation(sp[:], src, AF.Sigmoid)
                    nc.scalar.activation(sn[:], src, AF.Sigmoid, scale=-1.0)
                    rn = ap.tile([fs, Bc], F32, tag=f"rn{fi}", name=f"rn{fi}")
                    nc.vector.reciprocal(rn[:], sn[:])
                    ez = ap.tile([fs, 1, Bc], BF16, tag=f"expz{fi}", name=f"expz{fi}")
                    nc.vector.tensor_tensor(ez[:, 0, :], sp[:], rn[:], op=OP.mult)
                    expz.append(ez)

                # Z = sum_f expz  -> [1, NH, Bc] (4 broadcast copies), then
                # 1/Z broadcast to all partitions via PE: rbc[p, 4*32] = 1/Z_b
                for fi, (f0, fs) in enumerate(f_t):
                    nc.tensor.matmul(
                        sm[0:1, 6:10, :],
                        ones_f[fi][:],
                        expz[fi][:, :, :].broadcast_to([fs, NH, Bc]),
                        start=(fi == 0),
                        stop=(fi == NF - 1),
                    )
                recip4 = ap.tile([1, NH, Bc], F32, tag="recip4", name="recip4")
                nc.vector.reciprocal(recip4[:], sm[0:1, 6:10, :])
                rbc_ps = sm[:, 6:10, :]
                nc.tensor.matmul(
                    rbc_ps, onesf32[:], recip4[:], start=True, stop=True
                )

                # einsum: ctx[b,:] = sum_f expz[b,f] * encT[b,f,:]
                # 4 concurrent column strips; strip s round r -> sample 8s+r,
                # staged in blk[32s, r, :], gathered per strip by one DMA.
                blk = ap.tile([128, NR, C], BF16, tag="ctxblk", name="ctxblk")
                for r in range(NR):
                    eps = psE.tile([128, C], F32, tag="ein", name="ein")
                    for s in range(4):
                        b = 8 * s + r
                        for fi, (f0, fs) in enumerate(f_t):
                            nc.tensor.matmul(
                                eps[32 * s : 32 * s + 32, :],
                                expz[fi][:, 0, b : b + 1].broadcast_to([fs, 32]),
                                enc_sb[fi][s][:, r, :],
                                start=(fi == 0),
                                stop=(fi == NF - 1),
                                tile_position=(0, 32 * s),
                            )
                    if r % 2 == 0:
                        nc.scalar.activation(blk[:, r, :], eps[:], AF.Copy)
                    else:
                        nc.vector.tensor_copy(blk[:, r, :], eps[:])

                # vocab filler B: fills PE during gathers + scale chain
                if t >= 4:
                    emit_vocab(2)

                ctx_sb = ap.tile([Bc, C], BF16, tag="ctx", name="ctx")
                for s in range(4):
                    nc.gpsimd.dma_start(
                        ctx_sb[8 * s : 8 * s + 8, :],
                        blk[32 * s : 32 * s + 1, :, :],
                    )

                # sigmoid(gamma) in C-major, bf16
                gamS = ap.tile([128, NH, Bc], BF16, tag="gamS", name="gamS")
                nc.scalar.activation(gamS[:], gam_ps[:], AF.Sigmoid)

                # transpose ctx to [c, b] (bf16 PE transpose), then
                # ctxgT = ctxT * (1/Z) * sigmoid(gamma)   (2 DVE ops)
                ctT_ps = psS.tile([128, NH, Bc], BF16, tag="ctT", name="ctT")
                for j in range(NH):
                    nc.tensor.transpose(
                        ctT_ps[:, j, :], ctx_sb[:, j * 128 : (j + 1) * 128], eye_sb[:]
                    )
                ctn = ap.tile([128, NH, Bc], BF16, tag="ctn", name="ctn")
                nc.vector.tensor_tensor(
                    ctn[:, :, :], ctT_ps[:, :, :], gamS[:], op=OP.mult
                )
                ctxgT = ap.tile([128, NH, Bc], BF16, tag="ctxgT", name="ctxgT")
                nc.vector.tensor_tensor(ctxgT[:], ctn[:], rbc_ps, op=OP.mult)

                # comb: inp = relu(cx + WcT.T@ctxgT) -> bf16 [wv-part, b]
                cb_ps = psE.tile([128, C], F32, tag="ein", name="cb")
                for mi, (m0, ms) in enumerate(wv_t):
                    for ki in range(NH):
                        nc.tensor.matmul(
                            cb_ps[0:ms, mi * Bc : (mi + 1) * Bc],
                            WcT_sb[ki][:, m0 : m0 + ms],
                            ctxgT[:, ki, :],
                            start=(ki == 0),
                            stop=(ki == NH - 1),
                        )
                csum = ap.tile([128, NWV, Bc], F32, tag="csum", name="csum")
                inp_bf = ap.tile([128, NWV, Bc], BF16, tag="inp", name="inp")
                for mi, (m0, ms) in enumerate(wv_t):
                    nc.vector.tensor_tensor(
                        csum[0:ms, mi, :], cb_ps[0:ms, mi * Bc : (mi + 1) * Bc],
                        cx_sb[mi][:, tc0:tc1], op=OP.add,
                    )
                    nc.scalar.activation(
                        inp_bf[0:ms, mi, :], csum[0:ms, mi, :], AF.Relu
                    )

                # LSTM gates: [128, 16, Bc] psum
                rhs_list = [
                    (inp_bf[0:ks, ki, :], WihT_sb[ki]) for ki, (k0, ks) in enumerate(wv_t)
                ]
                if t > 0:
                    rhs_list += [(h_prev[ki], WhhT_sb[ki]) for ki in range(NH)]
                g_ps = psS.tile([128, 16, Bc], F32, tag="gates", name="gates")
                for m in range(16):
                    for j, (rhs, wt) in enumerate(rhs_list):
                        nc.tensor.matmul(
                            g_ps[:, m, :],
                            wt[:, m * 128 : (m + 1) * 128],
                            rhs,
                            start=(j == 0),
                            stop=(j == len(rhs_list) - 1),
                        )
                gsum = bp.tile([128, 16, Bc], F32, tag="gsum", name="gsum")
                nc.vector.tensor_tensor(gsum[:], g_ps[:], lstmb_sb[:], op=OP.add)
                nl = bp.tile([128, 16, Bc], F32, tag="nl", name="nl")
                for (a, b_, fn) in (
                    (0, 4, AF.Sigmoid), (4, 8, AF.Sigmoid),
                    (8, 12, AF.Tanh), (12, 16, AF.Sigmoid),
                ):
                    nc.scalar.activation(nl[:, a:b_, :], gsum[:, a:b_, :], fn)
                ig = ap.tile([128, 4, Bc], F32, tag="ig", name="ig")
                nc.vector.tensor_tensor(ig[:], nl[:, 0:4, :], nl[:, 8:12, :], op=OP.mult)
                c_new = st.tile([128, 4, Bc], F32, tag="c", name="c")
                if t > 0:
                    cf = ap.tile([128, 4, Bc], F32, tag="cf", name="cf")
                    nc.vector.tensor_tensor(cf[:], nl[:, 4:8, :], c_prev[:], op=OP.mult)
                    nc.vector.tensor_tensor(c_new[:], ig[:], cf[:], op=OP.add)
                else:
                    nc.vector.tensor_copy(c_new[:], ig[:])
                tanh_c = ap.tile([128, 4, Bc], F32, tag="tanh_c", name="tanh_c")
                nc.scalar.activation(tanh_c[:], c_new[:], AF.Tanh)
                nc.vector.tensor_tensor(
                    h_all[:, :, tc0:tc1], nl[:, 12:16, :], tanh_c[:], op=OP.mult
                )
                c_prev = c_new
                h_prev = [h_all[:, k, tc0:tc1] for k in range(NH)]

            if stage < 3:
                return nc

            # ---------------- vocab epilogue (remaining items) ----------------
            emit_vocab(len(vitems) - vpos[0])

    _split_multi_waits(nc)
    return nc


def _split_multi_waits(nc):
    """walrus' codegen accepts at most one sync wait per engine instruction
    in this environment; hoist extra waits onto same-engine NoOps placed
    immediately before the owning instruction."""
    for fn in nc.m.functions:
        for bb in fn.blocks:
            insts = bb.instructions
            out = []
            changed = False
            for inst in insts:
                si = inst.sync_info
                if si is not None and len(si.on_wait) > 1:
                    waits = list(si.on_wait)
                    for w in waits[:-1]:
                        out.append(
                            mybir.InstNoOp(
                                name=f"{inst.name}-w{len(out)}",
                                engine=inst.engine,
                                sync_info=mybir.SyncInfo(
                                    on_wait=[w], on_update=[]
                                ),
                            )
                        )
                    inst.sync_info = mybir.SyncInfo(
                        on_wait=[waits[-1]], on_update=list(si.on_update)
                    )
                    changed = True
                out.append(inst)
            if changed:
                bb.instructions = out


# ======================= host side =======================

def _bf16(x):
    return np.ascontiguousarray(np.asarray(x, dtype=ml_dtypes.bfloat16))


def prep_shared(inputs, Bc, T, V):
    """Weight-derived in_map entries (replicated across cores)."""
    attn_W = np.asarray(inputs["attn_W"], np.float32)
    comb_W = np.asarray(inputs["comb_W"], np.float32)
    gate_W = np.asarray(inputs["gate_W"], np.float32)
    sh = {
        "AxT": _bf16(attn_W[:, :WV].T),
        "AhT": _bf16(attn_W[:, WV:].T),
        "WxT": _bf16(comb_W[:, :WV].T),
        "WcT": _bf16(comb_W[:, WV:].T),
        "gateTa": _bf16(
            np.concatenate(
                [gate_W.T, np.asarray(inputs["gate_b"], np.float32)[None, :]], 0
            )
        ),
        "WihT": _bf16(np.asarray(inputs["lstm_Wih"]).T),
        "WhhT": _bf16(np.asarray(inputs["lstm_Whh"]).T),
        "vWT": _bf16(np.asarray(inputs["vocab_W"]).T[:, :V]),
        "attn_br": _bf16(np.asarray(inputs["attn_b"])[None, :]),
        "comb_br": _bf16(np.asarray(inputs["comb_b"])[None, :]),
        "eye": np.eye(Bc, dtype=ml_dtypes.bfloat16),
        "ones_col": np.ones((F, 1), dtype=ml_dtypes.bfloat16),
        "ones_row": np.ones((1, T * Bc), dtype=ml_dtypes.bfloat16),
        "ones_f32": np.ones((1, 128), dtype=np.float32),
    }
    bsum = (
        np.asarray(inputs["lstm_bih"], np.float32)
        + np.asarray(inputs["lstm_bhh"], np.float32)
    )
    bb = np.ascontiguousarray(bsum.reshape(16, 128).T)  # [128, 16]
    sh["lstm_bc"] = np.ascontiguousarray(
        np.broadcast_to(bb[:, :, None], (128, 16, Bc))
    ).astype(np.float32)
    return sh


def prep_core(inputs, core, Bc, T, V):
    """Batch-sharded in_map entries for one core."""
    b0, b1 = core * Bc, (core + 1) * Bc
    enc = np.asarray(inputs["encoding"], np.float32)[b0:b1]  # [Bc, C, F]
    wv = np.asarray(inputs["wordvecs"], np.float32)[b0:b1, :T]  # [Bc, T, WV]
    x_shift = np.concatenate(
        [np.zeros((Bc, 1, WV), np.float32), wv[:, :-1, :]], axis=1
    )
    return {
        "encT": _bf16(enc.transpose(0, 2, 1)),  # [Bc, F, C]
        "xT": _bf16(x_shift.transpose(2, 1, 0).reshape(WV, T * Bc)),
    }


_PROG_CACHE = {}
LAST_RESULT = None


def kernel(**inputs):
    global LAST_RESULT
    from concourse.bass_utils import run_bass_kernel_spmd

    Bc, T, V = B_FULL // N_CORES, T_FULL, V_FULL
    key = (Bc, T, V)
    if key not in _PROG_CACHE:
        _PROG_CACHE[key] = build_program(Bc, T, V)
    nc = _PROG_CACHE[key]

    shared = prep_shared(inputs, Bc, T, V)
    in_maps = [dict(shared, **prep_core(inputs, k, Bc, T, V)) for k in range(N_CORES)]
    res = run_bass_kernel_spmd(nc, in_maps, list(range(N_CORES)))
    LAST_RESULT = res

    parts = []
    for r in res.results:
        o = np.asarray(r["out"]).astype(np.float32).reshape(T, Bc, V).transpose(1, 0, 2)
        parts.append(o)
    out = np.concatenate(parts, axis=0)
    out = out + np.asarray(inputs["vocab_b"], np.float32)[None, None, :]
    return np.ascontiguousarray(out.astype(np.float32))


# revision 21
# speedup vs baseline: 1.0379x; 1.0379x over previous
"""CaptionNet Trainium2 kernel (8-core SPMD, data-parallel over batch).

Per core (batch shard Bc=32): attention-LSTM recurrence fully on-chip in a
feature-on-partition / batch-on-free layout, bf16 matmul operands with fp32
PSUM accumulation.

v2 design (vs the first working version):
- The vocab projection is interleaved into the recurrence as PE filler work
  (5 (m-chunk, v-chunk) slots per step from t=4 on), which removes the
  serial vocab tail and keeps the tensor engine continuously busy so it
  ramps to the full clock. Half of vocab_W.T stays resident in SBUF; the
  other half streams from HBM once per m-chunk.
- exp(z) is computed as sigmoid(z)/sigmoid(-z) (reciprocal on DVE), so every
  scalar-engine activation (sigmoid/tanh/relu/copy) lives in one activation
  table and the per-step 1.28us ACT_TABLE_LOAD swaps disappear.
- The softmax normalizer 1/Z is broadcast across partitions by the PE
  (ones-column x recip-row) and applied after the ctx transpose with one DVE
  multiply, keeping it off the softmax->einsum critical path.
- gamma is computed in C-major layout (16 LDWEIGHTS-light matmuls with N=32
  instead of 5 N=512 streams) and applied post-transpose.
- The attention einsum runs 4 concurrent PE column strips (8 rounds of 4
  samples); round extraction copies rotate over scalar/vector/gpsimd.
- Logits are written bf16 and upcast on the host.
"""

import numpy as np
import ml_dtypes

import concourse.bass as bass
import concourse.tile as tile
import concourse.mybir as mybir

BF16 = mybir.dt.bfloat16
F32 = mybir.dt.float32
AF = mybir.ActivationFunctionType
OP = mybir.AluOpType

# Problem constants (full size)
B_FULL, T_FULL, H, WV, F, C, V_FULL = 256, 20, 512, 301, 196, 512, 9871
N_CORES = 8
F_HI = 128
F_LO = F - F_HI  # 68

VCHUNK = 512
N_VRES = 10  # resident v-chunks (first N_VRES*512 vocab columns stay in SBUF)


def _tiles(total, step=128):
    return [(i, min(step, total - i)) for i in range(0, total, step)]


def build_program(Bc=32, T=20, V=V_FULL, stage=99):
    TB = Bc * T
    NG = Bc // 8  # sample groups of 8 (= einsum strips)
    NR = Bc // 4  # einsum rounds per step (4 strips)
    nc = bass.Bass()

    # ---------------- DRAM I/O (per-core) ----------------
    encT_d = nc.dram_tensor("encT", [Bc, F, C], BF16, kind="ExternalInput")
    xT_d = nc.dram_tensor("xT", [WV, TB], BF16, kind="ExternalInput")
    AxT_d = nc.dram_tensor("AxT", [WV, F], BF16, kind="ExternalInput")
    AhT_d = nc.dram_tensor("AhT", [H, F], BF16, kind="ExternalInput")
    WxT_d = nc.dram_tensor("WxT", [WV, WV], BF16, kind="ExternalInput")
    WcT_d = nc.dram_tensor("WcT", [C, WV], BF16, kind="ExternalInput")
    gateTa_d = nc.dram_tensor("gateTa", [H + 1, C], BF16, kind="ExternalInput")
    WihT_d = nc.dram_tensor("WihT", [WV, 4 * H], BF16, kind="ExternalInput")
    WhhT_d = nc.dram_tensor("WhhT", [H, 4 * H], BF16, kind="ExternalInput")
    vWT_d = nc.dram_tensor("vWT", [H, V], BF16, kind="ExternalInput")
    attnb_d = nc.dram_tensor("attn_br", [1, F], BF16, kind="ExternalInput")
    combb_d = nc.dram_tensor("comb_br", [1, WV], BF16, kind="ExternalInput")
    lstmb_d = nc.dram_tensor("lstm_bc", [128, 16, Bc], F32, kind="ExternalInput")
    eye_d = nc.dram_tensor("eye", [Bc, Bc], BF16, kind="ExternalInput")
    onesc_d = nc.dram_tensor("ones_col", [F, 1], BF16, kind="ExternalInput")
    onesr_d = nc.dram_tensor("ones_row", [1, TB], BF16, kind="ExternalInput")
    onesf_d = nc.dram_tensor("ones_f32", [1, 128], F32, kind="ExternalInput")
    out_d = nc.dram_tensor("out", [TB, V], BF16, kind="ExternalOutput")

    wv_t = _tiles(WV)   # [(0,128),(128,128),(256,45)]
    h_t = _tiles(H)     # 4 x 128
    f_t = [(0, F_HI), (F_HI, F_LO)]
    NWV, NH, NF = len(wv_t), len(h_t), len(f_t)
    n_mv = _tiles(TB)   # vocab m-chunks along T*Bc (5 x 128)
    v_ch = _tiles(V, VCHUNK)  # vocab n-chunks (20)

    with tile.TileContext(nc) as tc:
        with (
            tc.tile_pool(name="w", bufs=1) as wp,
            tc.tile_pool(name="act", bufs=2) as ap,
            tc.tile_pool(name="big", bufs=1) as bp,
            tc.tile_pool(name="st", bufs=2) as st,
            tc.tile_pool(name="vs", bufs=2) as vsp,
            tc.tile_pool(name="vo", bufs=2) as vp,
            tc.tile_pool(name="psE", bufs=3, space="PSUM") as psE,
            tc.tile_pool(name="psV", bufs=2, space="PSUM") as psV,
            tc.tile_pool(name="psS", bufs=1, space="PSUM") as psS,
        ):

            # ---------------- resident loads ----------------
            def load_ktiles(dram, ktiles, ncols, dt, name):
                out = []
                for ki, (k0, ks) in enumerate(ktiles):
                    tl = wp.tile([ks, ncols], dt, tag=f"{name}{ki}", name=f"{name}{ki}")
                    nc.sync.dma_start(tl[:], dram[k0 : k0 + ks, :])
                    out.append(tl)
                return out

            ones_f = load_ktiles(onesc_d, f_t, 1, BF16, "ones")
            ones1 = wp.tile([1, TB], BF16, tag="onesr", name="onesr")
            nc.sync.dma_start(ones1[:], onesr_d[:])
            onesf32 = wp.tile([1, 128], F32, tag="onesf32", name="onesf32")
            nc.sync.dma_start(onesf32[:], onesf_d[:])
            eye_sb = wp.tile([Bc, Bc], BF16, tag="eye", name="eye")
            nc.sync.dma_start(eye_sb[:], eye_d[:])

            xT_sb = load_ktiles(xT_d, wv_t, TB, BF16, "xT")
            AxT_sb = load_ktiles(AxT_d, wv_t, F, BF16, "AxT")
            AhT_sb = load_ktiles(AhT_d, h_t, F, BF16, "AhT")
            WxT_sb = load_ktiles(WxT_d, wv_t, WV, BF16, "WxT")
            WcT_sb = load_ktiles(WcT_d, h_t, WV, BF16, "WcT")
            gateT_sb = load_ktiles(gateTa_d, h_t, C, BF16, "gateT")
            gateB_sb = wp.tile([1, C], BF16, tag="gateB", name="gateB")
            nc.sync.dma_start(gateB_sb[:], gateTa_d[H : H + 1, :])
            WihT_sb = load_ktiles(WihT_d, wv_t, 4 * H, BF16, "WihT")
            WhhT_sb = load_ktiles(WhhT_d, h_t, 4 * H, BF16, "WhhT")
            attnb_sb = wp.tile([1, F], BF16, tag="attnbr", name="attnbr")
            nc.sync.dma_start(attnb_sb[:], attnb_d[:])
            combb_sb = wp.tile([1, WV], BF16, tag="combbr", name="combbr")
            nc.sync.dma_start(combb_sb[:], combb_d[:])
            lstmb_sb = wp.tile([128, 16, Bc], F32, tag="lstmb", name="lstmb")
            nc.sync.dma_start(lstmb_sb[:], lstmb_d[:])

            # encoding, split per f-tile x sample-group for fine-grained deps
            encT_r = encT_d.rearrange("b f c -> f b c")
            enc_sb = [[None] * NG for _ in range(NF)]
            for fi, (f0, fs) in enumerate(f_t):
                for g in range(NG):
                    e = wp.tile([fs, 8, C], BF16, tag=f"enc{fi}_{g}", name=f"enc{fi}_{g}")
                    nc.sync.dma_start(e[:], encT_r[f0 : f0 + fs, 8 * g : 8 * g + 8, :])
                    enc_sb[fi][g] = e

            # resident half of the vocab weights [128, NH, N_VRES*VCHUNK]
            VRES = N_VRES * VCHUNK
            vres = wp.tile([128, NH, VRES], BF16, tag="vres", name="vres")
            for ki in range(NH):
                nc.sync.dma_start(
                    vres[:, ki, :], vWT_d[128 * ki : 128 * ki + 128, 0:VRES]
                )

            h_all = bp.tile([128, NH, TB], BF16, tag="h_all", name="h_all")

            if stage < 1:
                return nc

            # ---------------- x-precomputes: zx = x@Ax.T+b ; cx = x@Wx.T+b ----
            def precompute(weights, mtiles, dst_tiles, bias_row):
                for mi, (m0, ms) in enumerate(mtiles):
                    for n0 in range(0, TB, 512):
                        nn = min(512, TB - n0)
                        pps = psE.tile([128, 512], F32, tag="ein", name="pre")
                        for ki in range(len(weights)):
                            nc.tensor.matmul(
                                pps[0:ms, 0:nn],
                                weights[ki][:, m0 : m0 + ms],
                                xT_sb[ki][:, n0 : n0 + nn],
                                start=(ki == 0),
                                stop=False,
                            )
                        nc.tensor.matmul(
                            pps[0:ms, 0:nn],
                            bias_row[:, m0 : m0 + ms],
                            ones1[:, n0 : n0 + nn],
                            start=False,
                            stop=True,
                        )
                        nc.vector.tensor_copy(
                            dst_tiles[mi][:, n0 : n0 + nn], pps[0:ms, 0:nn]
                        )

            zx_sb = [
                bp.tile([fs, TB], BF16, tag=f"zx{fi}", name=f"zx{fi}")
                for fi, (f0, fs) in enumerate(f_t)
            ]
            precompute(AxT_sb, f_t, zx_sb, attnb_sb)
            cx_sb = [
                bp.tile([ms, TB], BF16, tag=f"cx{mi}", name=f"cx{mi}")
                for mi, (m0, ms) in enumerate(wv_t)
            ]
            precompute(WxT_sb, wv_t, cx_sb, combb_sb)

            if stage < 2:
                return nc

            # ---------------- vocab projection slots ----------------
            # Work item (j, v): out[m-chunk j, v-chunk v]. m-chunk j (tokens
            # 128j..128j+128 = steps 4j..4j+4) is ready after step 4j+3, so
            # emitting items 5(t-4)..5(t-4)+5 at step t keeps every item legal.
            vcopy_rr = [0]

            def vocab_slot(j, v):
                m0, ms = n_mv[j]
                n0, nn = v_ch[v]
                if v < N_VRES:
                    vw_ap = vres[:, :, n0 : n0 + nn]
                else:
                    vst = vsp.tile([128, NH, VCHUNK], BF16, tag="vs", name="vs")
                    for ki in range(NH):
                        nc.sync.dma_start(
                            vst[:, ki, 0:nn],
                            vWT_d[128 * ki : 128 * ki + 128, n0 : n0 + nn],
                        )
                    vw_ap = vst[:, :, :]
                vps = psV.tile([128, 512], F32, tag="voc", name="voc")
                for ki in range(NH):
                    nc.tensor.matmul(
                        vps[0:ms, 0:nn],
                        h_all[:, ki, m0 : m0 + ms],
                        vw_ap[:, ki, 0:nn],
                        start=(ki == 0),
                        stop=(ki == NH - 1),
                    )
                vo = vp.tile([128, 512], BF16, tag="vout", name="vout")
                r = vcopy_rr[0] % 2
                vcopy_rr[0] += 1
                if r == 0:
                    nc.scalar.activation(vo[0:ms, 0:nn], vps[0:ms, 0:nn], AF.Copy)
                else:
                    nc.vector.tensor_copy(vo[0:ms, 0:nn], vps[0:ms, 0:nn])
                q = (nc.sync, nc.scalar)[vcopy_rr[0] % 2]
                q.dma_start(out_d[m0 : m0 + ms, n0 : n0 + nn], vo[0:ms, 0:nn])

            vitems = [(j, v) for j in range(len(n_mv)) for v in range(len(v_ch))]
            vpos = [0]  # next unemitted item

            def emit_vocab(k):
                for _ in range(k):
                    if vpos[0] < len(vitems):
                        j, v = vitems[vpos[0]]
                        vocab_slot(j, v)
                        vpos[0] += 1

            # ---------------- recurrence ----------------
            c_prev = None
            h_prev = None
            for t in range(T):
                tc0, tc1 = t * Bc, (t + 1) * Bc

                # vocab filler A: runs on PE while the previous step's LSTM
                # elementwise chain produces h_{t-1} (emitted first so the PE
                # is not blocked behind zh's wait).
                if t >= 4:
                    emit_vocab(2)

                # one packed PSUM bank for the step's small tensors:
                # [0:2]=zh, [2:6]=gamT, [6:10]=se4 (row 0) then rbc (all rows,
                # written only after se4 is consumed)
                sm = psS.tile([128, 10, Bc], F32, tag="small", name="sm")
                zh_ps = sm[:, 0:2, :]
                gam_ps = sm[:, 2:6, :]

                # attention z, h-part
                if t > 0:
                    for mi, (m0, ms) in enumerate(f_t):
                        for ki in range(NH):
                            nc.tensor.matmul(
                                zh_ps[0:ms, mi, :],
                                AhT_sb[ki][:, m0 : m0 + ms],
                                h_prev[ki],
                                start=(ki == 0),
                                stop=(ki == NH - 1),
                            )

                # gamma pre-activation, C-major: gamT[c, b]
                for m in range(NH):
                    if t > 0:
                        for ki in range(NH):
                            nc.tensor.matmul(
                                gam_ps[:, m, :],
                                gateT_sb[ki][:, 128 * m : 128 * m + 128],
                                h_prev[ki],
                                start=(ki == 0),
                                stop=False,
                            )
                    nc.tensor.matmul(
                        gam_ps[:, m, :],
                        gateB_sb[:, 128 * m : 128 * m + 128],
                        ones1[:, 0:Bc],
                        start=(t == 0),
                        stop=True,
                    )
                del m

                # vocab filler B: fills the PE while scalar/DVE run the
                # sigmoid-trick exp chain below.
                if t >= 4:
                    emit_vocab(1)

                # expz = exp(zx + zh) via sigmoid(z)/sigmoid(-z); bf16 [fs,1,Bc]
                expz = []
                for fi, (f0, fs) in enumerate(f_t):
                    if t > 0:
                        zs = ap.tile([fs, Bc], F32, tag=f"zs{fi}", name=f"zs{fi}")
                        nc.vector.tensor_tensor(
                            zs[:], zh_ps[0:fs, fi, :],
                            zx_sb[fi][:, tc0:tc1], op=OP.add,
                        )
                        src = zs[:, :]
                    else:
                        src = zx_sb[fi][:, tc0:tc1]
                    sp = ap.tile([fs, Bc], F32, tag=f"sp{fi}", name=f"sp{fi}")
                    sn = ap.tile([fs, Bc], F32, tag=f"sn{fi}", name=f"sn{fi}")
                    nc.scalar.activation(sp[:], src, AF.Sigmoid)
                    nc.scalar.activation(sn[:], src, AF.Sigmoid, scale=-1.0)
                    rn = ap.tile([fs, Bc], F32, tag=f"rn{fi}", name=f"rn{fi}")
                    nc.vector.reciprocal(rn[:], sn[:])
                    ez = ap.tile([fs, 1, Bc], BF16, tag=f"expz{fi}", name=f"expz{fi}")
                    nc.vector.tensor_tensor(ez[:, 0, :], sp[:], rn[:], op=OP.mult)
                    expz.append(ez)

                # Z = sum_f expz  -> [1, NH, Bc] (4 broadcast copies), then
                # 1/Z broadcast to all partitions via PE: rbc[p, 4*32] = 1/Z_b
                for fi, (f0, fs) in enumerate(f_t):
                    nc.tensor.matmul(
                        sm[0:1, 6, :],
                        ones_f[fi][:],
                        expz[fi][:, 0, :],
                        start=(fi == 0),
                        stop=(fi == NF - 1),
                    )
                recip = ap.tile([1, 1, Bc], F32, tag="recip", name="recip")
                nc.vector.reciprocal(recip[:, 0, :], sm[0:1, 6, :])
                rbc_ps = sm[:, 6:10, :]
                nc.tensor.matmul(
                    rbc_ps, onesf32[:],
                    recip[:, :, :].broadcast_to([1, NH, Bc]),
                    start=True, stop=True,
                )

                # einsum: ctx[b,:] = sum_f expz[b,f] * encT[b,f,:]
                # 4 concurrent column strips; strip s round r -> sample 8s+r,
                # staged in blk half-tiles [32s, r%4, :]; per-strip gathers
                # for rounds 0-3 overlap einsum rounds 4-7, spread over 4
                # DMA queues.
                NRH = NR // 2
                blkA = ap.tile([128, NRH, C], BF16, tag="ctxblkA", name="ctxblkA")
                blkB = ap.tile([128, NRH, C], BF16, tag="ctxblkB", name="ctxblkB")
                ctx_sb = ap.tile([Bc, C], BF16, tag="ctx", name="ctx")
                qs = (nc.gpsimd, nc.sync, nc.gpsimd, nc.scalar)
                for r in range(NR):
                    blk = blkA if r < NRH else blkB
                    eps = psE.tile([128, C], F32, tag="ein", name="ein")
                    for s in range(4):
                        b = 8 * s + r
                        for fi, (f0, fs) in enumerate(f_t):
                            nc.tensor.matmul(
                                eps[32 * s : 32 * s + 32, :],
                                expz[fi][:, 0, b : b + 1].broadcast_to([fs, 32]),
                                enc_sb[fi][s][:, r, :],
                                start=(fi == 0),
                                stop=(fi == NF - 1),
                                tile_position=(0, 32 * s),
                            )
                    if r % 2 == 0:
                        nc.scalar.activation(blk[:, r % NRH, :], eps[:], AF.Copy)
                    else:
                        nc.vector.tensor_copy(blk[:, r % NRH, :], eps[:])
                    if r == NRH - 1:
                        for s in range(4):
                            qs[s].dma_start(
                                ctx_sb[8 * s : 8 * s + NRH, :],
                                blkA[32 * s : 32 * s + 1, :, :],
                            )
                for s in range(4):
                    qs[s].dma_start(
                        ctx_sb[8 * s + NRH : 8 * s + 8, :],
                        blkB[32 * s : 32 * s + 1, :, :],
                    )

                # vocab filler C: fills PE during the blkB gathers + scale
                if t >= 4:
                    emit_vocab(1)

                # sigmoid(gamma) in C-major, bf16
                gamS = ap.tile([128, NH, Bc], BF16, tag="gamS", name="gamS")
                nc.scalar.activation(gamS[:], gam_ps[:], AF.Sigmoid)

                # transpose ctx to [c, b] (bf16 PE transpose), then
                # ctxgT = ctxT * (1/Z) * sigmoid(gamma)   (2 DVE ops)
                ctT_ps = psS.tile([128, NH, Bc], BF16, tag="ctT", name="ctT")
                for j in range(NH):
                    nc.tensor.transpose(
                        ctT_ps[:, j, :], ctx_sb[:, j * 128 : (j + 1) * 128], eye_sb[:]
                    )
                ctn = ap.tile([128, NH, Bc], BF16, tag="ctn", name="ctn")
                nc.vector.tensor_tensor(
                    ctn[:, :, :], ctT_ps[:, :, :], gamS[:], op=OP.mult
                )
                ctxgT = ap.tile([128, NH, Bc], BF16, tag="ctxgT", name="ctxgT")
                nc.vector.tensor_tensor(ctxgT[:], ctn[:], rbc_ps, op=OP.mult)

                # comb: inp = relu(cx + WcT.T@ctxgT) -> bf16 [wv-part, b]
                cb_ps = psE.tile([128, C], F32, tag="ein", name="cb")
                for mi, (m0, ms) in enumerate(wv_t):
                    for ki in range(NH):
                        nc.tensor.matmul(
                            cb_ps[0:ms, mi * Bc : (mi + 1) * Bc],
                            WcT_sb[ki][:, m0 : m0 + ms],
                            ctxgT[:, ki, :],
                            start=(ki == 0),
                            stop=(ki == NH - 1),
                        )
                csum = ap.tile([128, NWV, Bc], F32, tag="csum", name="csum")
                inp_bf = ap.tile([128, NWV, Bc], BF16, tag="inp", name="inp")
                for mi, (m0, ms) in enumerate(wv_t):
                    nc.vector.tensor_tensor(
                        csum[0:ms, mi, :], cb_ps[0:ms, mi * Bc : (mi + 1) * Bc],
                        cx_sb[mi][:, tc0:tc1], op=OP.add,
                    )
                    nc.scalar.activation(
                        inp_bf[0:ms, mi, :], csum[0:ms, mi, :], AF.Relu
                    )

                # vocab filler D: fills PE during the relu chain
                if t >= 4:
                    emit_vocab(1)

                # LSTM gates: [128, 16, Bc] psum
                rhs_list = [
                    (inp_bf[0:ks, ki, :], WihT_sb[ki]) for ki, (k0, ks) in enumerate(wv_t)
                ]
                if t > 0:
                    rhs_list += [(h_prev[ki], WhhT_sb[ki]) for ki in range(NH)]
                g_ps = psS.tile([128, 16, Bc], F32, tag="gates", name="gates")
                for m in range(16):
                    for j, (rhs, wt) in enumerate(rhs_list):
                        nc.tensor.matmul(
                            g_ps[:, m, :],
                            wt[:, m * 128 : (m + 1) * 128],
                            rhs,
                            start=(j == 0),
                            stop=(j == len(rhs_list) - 1),
                        )
                gsum = bp.tile([128, 16, Bc], F32, tag="gsum", name="gsum")
                nc.vector.tensor_tensor(gsum[:], g_ps[:], lstmb_sb[:], op=OP.add)
                nl = bp.tile([128, 16, Bc], F32, tag="nl", name="nl")
                for (a, b_, fn) in (
                    (0, 4, AF.Sigmoid), (4, 8, AF.Sigmoid),
                    (8, 12, AF.Tanh), (12, 16, AF.Sigmoid),
                ):
                    nc.scalar.activation(nl[:, a:b_, :], gsum[:, a:b_, :], fn)
                ig = ap.tile([128, 4, Bc], F32, tag="ig", name="ig")
                nc.vector.tensor_tensor(ig[:], nl[:, 0:4, :], nl[:, 8:12, :], op=OP.mult)
                c_new = st.tile([128, 4, Bc], F32, tag="c", name="c")
                if t > 0:
                    cf = ap.tile([128, 4, Bc], F32, tag="cf", name="cf")
                    nc.vector.tensor_tensor(cf[:], nl[:, 4:8, :], c_prev[:], op=OP.mult)
                    nc.vector.tensor_tensor(c_new[:], ig[:], cf[:], op=OP.add)
                else:
                    nc.vector.tensor_copy(c_new[:], ig[:])
                tanh_c = ap.tile([128, 4, Bc], F32, tag="tanh_c", name="tanh_c")
                nc.scalar.activation(tanh_c[:], c_new[:], AF.Tanh)
                nc.vector.tensor_tensor(
                    h_all[:, :, tc0:tc1], nl[:, 12:16, :], tanh_c[:], op=OP.mult
                )
                c_prev = c_new
                h_prev = [h_all[:, k, tc0:tc1] for k in range(NH)]

            if stage < 3:
                return nc

            # ---------------- vocab epilogue (remaining items) ----------------
            emit_vocab(len(vitems) - vpos[0])

    _split_multi_waits(nc)
    return nc


def _split_multi_waits(nc):
    """walrus' codegen accepts at most one sync wait per engine instruction
    in this environment; hoist extra waits onto same-engine NoOps placed
    immediately before the owning instruction."""
    for fn in nc.m.functions:
        for bb in fn.blocks:
            insts = bb.instructions
            out = []
            changed = False
            for inst in insts:
                si = inst.sync_info
                if si is not None and len(si.on_wait) > 1:
                    waits = list(si.on_wait)
                    for w in waits[:-1]:
                        out.append(
                            mybir.InstNoOp(
                                name=f"{inst.name}-w{len(out)}",
                                engine=inst.engine,
                                sync_info=mybir.SyncInfo(
                                    on_wait=[w], on_update=[]
                                ),
                            )
                        )
                    inst.sync_info = mybir.SyncInfo(
                        on_wait=[waits[-1]], on_update=list(si.on_update)
                    )
                    changed = True
                out.append(inst)
            if changed:
                bb.instructions = out


# ======================= host side =======================

def _bf16(x):
    return np.ascontiguousarray(np.asarray(x, dtype=ml_dtypes.bfloat16))


def prep_shared(inputs, Bc, T, V):
    """Weight-derived in_map entries (replicated across cores)."""
    attn_W = np.asarray(inputs["attn_W"], np.float32)
    comb_W = np.asarray(inputs["comb_W"], np.float32)
    gate_W = np.asarray(inputs["gate_W"], np.float32)
    sh = {
        "AxT": _bf16(attn_W[:, :WV].T),
        "AhT": _bf16(attn_W[:, WV:].T),
        "WxT": _bf16(comb_W[:, :WV].T),
        "WcT": _bf16(comb_W[:, WV:].T),
        "gateTa": _bf16(
            np.concatenate(
                [gate_W.T, np.asarray(inputs["gate_b"], np.float32)[None, :]], 0
            )
        ),
        "WihT": _bf16(np.asarray(inputs["lstm_Wih"]).T),
        "WhhT": _bf16(np.asarray(inputs["lstm_Whh"]).T),
        "vWT": _bf16(np.asarray(inputs["vocab_W"]).T[:, :V]),
        "attn_br": _bf16(np.asarray(inputs["attn_b"])[None, :]),
        "comb_br": _bf16(np.asarray(inputs["comb_b"])[None, :]),
        "eye": np.eye(Bc, dtype=ml_dtypes.bfloat16),
        "ones_col": np.ones((F, 1), dtype=ml_dtypes.bfloat16),
        "ones_row": np.ones((1, T * Bc), dtype=ml_dtypes.bfloat16),
        "ones_f32": np.ones((1, 128), dtype=np.float32),
    }
    bsum = (
        np.asarray(inputs["lstm_bih"], np.float32)
        + np.asarray(inputs["lstm_bhh"], np.float32)
    )
    bb = np.ascontiguousarray(bsum.reshape(16, 128).T)  # [128, 16]
    sh["lstm_bc"] = np.ascontiguousarray(
        np.broadcast_to(bb[:, :, None], (128, 16, Bc))
    ).astype(np.float32)
    return sh


def prep_core(inputs, core, Bc, T, V):
    """Batch-sharded in_map entries for one core."""
    b0, b1 = core * Bc, (core + 1) * Bc
    enc = np.asarray(inputs["encoding"], np.float32)[b0:b1]  # [Bc, C, F]
    wv = np.asarray(inputs["wordvecs"], np.float32)[b0:b1, :T]  # [Bc, T, WV]
    x_shift = np.concatenate(
        [np.zeros((Bc, 1, WV), np.float32), wv[:, :-1, :]], axis=1
    )
    return {
        "encT": _bf16(enc.transpose(0, 2, 1)),  # [Bc, F, C]
        "xT": _bf16(x_shift.transpose(2, 1, 0).reshape(WV, T * Bc)),
    }


_PROG_CACHE = {}
LAST_RESULT = None


def kernel(**inputs):
    global LAST_RESULT
    from concourse.bass_utils import run_bass_kernel_spmd

    Bc, T, V = B_FULL // N_CORES, T_FULL, V_FULL
    key = (Bc, T, V)
    if key not in _PROG_CACHE:
        _PROG_CACHE[key] = build_program(Bc, T, V)
    nc = _PROG_CACHE[key]

    shared = prep_shared(inputs, Bc, T, V)
    in_maps = [dict(shared, **prep_core(inputs, k, Bc, T, V)) for k in range(N_CORES)]
    res = run_bass_kernel_spmd(nc, in_maps, list(range(N_CORES)))
    LAST_RESULT = res

    parts = []
    for r in res.results:
        o = np.asarray(r["out"]).astype(np.float32).reshape(T, Bc, V).transpose(1, 0, 2)
        parts.append(o)
    out = np.concatenate(parts, axis=0)
    out = out + np.asarray(inputs["vocab_b"], np.float32)[None, None, :]
    return np.ascontiguousarray(out.astype(np.float32))
